# revision 39
# baseline (speedup 1.0000x reference)
"""DeepSeekMoE kernel for 8 TRN2 NeuronCores.

Sharding: quarter-expert-parallel. Each routed expert's FFN is split in
four along the hidden (H) axis across four cores; experts are paired by
sorted token count (1st with 2nd, 3rd with 4th, ...) so each of the four
SPMD slot-classes carries two near-equal experts and its cap is the pair
max — per-core routed matmul columns drop to 128*sum(slot caps) (~265k),
within ~1% of the theoretical balance floor. Each core also owns a 1/8
H-shard of the shared expert (tensor-parallel).

The tiny gate (sigmoid + top-2 over E=8) runs on host; tokens are gathered
per expert, padded to ntc*tsz equal 16B-aligned chunks (SPMD: one program
for all 8 cores), and shipped pre-transposed so every device-side matmul
contracts over the partition dimension. Each core returns
  ye0..ye3: [D, cap_j]  quarter-expert PARTIAL outputs, combine-weighted
  sh:       [T, D]      shared-expert partial (its H-shard)
Host scatter-adds the 4 quarter partials per expert by token index and
sums the 8 sh partials — the output gather performs the MoE combine; no
on-device collectives needed.

Timing model (measured): the PE streams bf16 matmuls at ~0.42 ns/col when
all INPUT streams ride the software-DGE path (gpsimd dma_start) — the
hardware-DGE input path drags the sustained PE clock from ~2.37GHz down
to ~1.97GHz (P0 power state). Output streams on Sync/Scalar HW-DGE do not
trigger it. LDWEIGHTS is fully hidden. The schedule therefore aims for a
single gapless matmul stream:
  - warmup matmuls start on a gpsimd-memset scratch slice (~7.5us, right
    after the framework preamble) and keep-alive matmuls gated on each
    early transfer hold the HAM clock gate open until real data lands.
  - xe ships token-chunk-major and the first slot's first w1 h-block as
    its own 0.25MB tensor, so the first real matmul needs ~0.78MB landed.
  - every next phase's head inputs (next slot's xe, wr, first w2 slabs,
    first xt chunk) are triggered a few slabs EARLY — the in-order input
    queue otherwise delivers them just-in-time and the PE stalls ~1-2us
    at each transition.
  - the last two tile casts alternate scalar/vector and the final sh tile
    DMA is split across the Sync and Scalar queues.

Compute dtype: bf16 operands, fp32 PSUM accumulation (rel err ~4e-3).
fp8 was evaluated and rejected: e4m3's 3-bit mantissa gives ~5e-2 rel err
on these random-walk GEMM sums (no sqrt(K) averaging), over the 2e-2 gate.
"""

import hashlib
import sys

sys.path.insert(0, "/opt/trn_rl_repo")

import numpy as np
import ml_dtypes

import concourse.bass as bass
import concourse.bacc as bacc
import concourse.mybir as mybir
import concourse.tile as tile
from concourse.bass_utils import run_bass_kernel_spmd

BF16 = ml_dtypes.bfloat16
F32 = np.float32

T, D, E, TOP_K, H = 2048, 1024, 8, 2, 4096
HQ = H // 4          # quarter-expert hidden
HS = H // 8          # shared-expert hidden shard per core
KD = D // 128        # 8  k-chunks over D
KHQ = HQ // 128      # 8  k-chunks over a quarter-expert
KHS = HS // 128      # 4  k-chunks over the shared shard
NSLOT = 4
N_CORES = 8

_DT = mybir.dt.bfloat16
_cache: dict = {}
_wcache: dict = {}


def _slotdims(n):
    """Equal token-chunks: (ntc, tsz) with ntc*tsz >= n, tsz <= 512.
    tsz is a multiple of 8 so bf16 moving-operand rows stay 16B-aligned —
    unaligned rows (e.g. tsz=270) measurably slow the PE stream by ~20%."""
    n = max(64, int(n))
    ntc = -(-n // 512)
    tsz = -(-n // (8 * ntc)) * 8
    return ntc, tsz


def _build(dims):
    """Build + finalize the SPMD device program for a 4-tuple of (ntc,tsz)."""
    nc = bacc.Bacc("TRN2", target_bir_lowering=False, debug=False)

    caps = tuple(ntc * tsz for ntc, tsz in dims)
    xe_d, w1_d, w2_d, b1_d, wr_d, ye_d = {}, {}, {}, {}, {}, {}
    for j, ((ntc, tsz), cap) in enumerate(zip(dims, caps)):
        xe_d[j] = nc.dram_tensor(f"xe{j}", [ntc, 128, KD, tsz], _DT, kind="ExternalInput")
        w1_d[j] = nc.dram_tensor(f"w1{j}", [KHQ // 2, 128, KD, 256], _DT, kind="ExternalInput")
        w2_d[j] = nc.dram_tensor(f"w2{j}", [4, 128, KHQ, 256], _DT, kind="ExternalInput")
        b1_d[j] = nc.dram_tensor(f"b1{j}", [128, KHQ], mybir.dt.float32, kind="ExternalInput")
        wr_d[j] = nc.dram_tensor(f"wr{j}", [128, cap], mybir.dt.float32, kind="ExternalInput")
        ye_d[j] = nc.dram_tensor(f"ye{j}", [D, cap], _DT, kind="ExternalOutput")
    # slot-0 h-block 0 duplicated as its own small tensor: the first real
    # matmul then needs only 0.25MB of weights + one xe chunk
    w1f_d = nc.dram_tensor("w1f", [128, KD, 128], _DT, kind="ExternalInput")
    xt_d = nc.dram_tensor("xt", [4, 128, KD, 512], _DT, kind="ExternalInput")
    ws1_d = nc.dram_tensor("ws1", [128, KD, HS], _DT, kind="ExternalInput")
    ws2_d = nc.dram_tensor("ws2", [128, KHS, D], _DT, kind="ExternalInput")
    bs1_d = nc.dram_tensor("bs1c", [128, KHS], mybir.dt.float32, kind="ExternalInput")
    sh_d = nc.dram_tensor("sh", [T, D], _DT, kind="ExternalOutput")

    gelu = mybir.ActivationFunctionType.Gelu

    with tile.TileContext(nc) as tc:
        with (
            tc.tile_pool(name="resident", bufs=1) as rpool,
            tc.tile_pool(name="w1s", bufs=4) as w1pool,
            tc.tile_pool(name="w2s", bufs=4) as w2pool,
            tc.tile_pool(name="xts", bufs=3) as xtpool,
            tc.tile_pool(name="psum", bufs=7, space="PSUM") as pspool,
            tc.tile_pool(name="wpsum", bufs=1, space="PSUM") as wpspool,
            tc.tile_pool(name="outs", bufs=6) as opool,
        ):
            # ---- PE warmup: dummy matmuls while the first DMAs are in
            # flight, so the HAM clock gate is released for the real work.
            scratch = rpool.tile([128, 512], _DT)
            nc.gpsimd.memset(scratch[:, 0:128], 0.0)
            nc.vector.memset(scratch[:, 128:512], 0.0)
            wps = wpspool.tile([128, 512], mybir.dt.float32)
            for _ in range(2):
                nc.tensor.matmul(wps[:, 0:128], scratch[:, 0:128], scratch[:, 0:128], start=True, stop=True)
            for _ in range(9):
                nc.tensor.matmul(wps[:], scratch[:, 0:128], scratch[:], start=True, stop=True)
            for _ in range(3):
                nc.tensor.matmul(wps[:, 0:128], scratch[:, 0:128], scratch[:, 0:128], start=True, stop=True)

            # ---- slot-0-critical loads, in first-consumption order, all on
            # the gpsimd (software-DGE) queue
            xe_sb = {j: [] for j in range(NSLOT)}
            wr_sb, b1_sb, hT = {}, {}, {}
            (ntc0, tsz0) = dims[0]
            for i in range(ntc0):
                xe_sb[0].append(rpool.tile([128, KD, tsz0], _DT, name=f"xe0_{i}"))
            nc.gpsimd.dma_start(xe_sb[0][0][:], xe_d[0][0])
            w1f = rpool.tile([128, KD, 128], _DT, name="w1f")
            nc.gpsimd.dma_start(w1f[:], w1f_d[:])
            for i in range(1, ntc0):
                nc.gpsimd.dma_start(xe_sb[0][i][:], xe_d[0][i])
            b1_sb[0] = rpool.tile([128, KHQ], mybir.dt.float32, name="b1_0")
            nc.gpsimd.dma_start(b1_sb[0][:], b1_d[0][:])

            # keep-alive touches hold the HAM gate open until data lands
            nc.tensor.matmul(wps[:, 0:128], xe_sb[0][0][:, 0, 0:128],
                             scratch[:, 0:128], start=True, stop=True)
            nc.tensor.matmul(wps[:, 0:128], w1f[:, 0, :],
                             scratch[:, 0:128], start=True, stop=True)
            if ntc0 > 1:
                nc.tensor.matmul(wps[:, 0:128], xe_sb[0][1][:, 0, 0:128],
                                 scratch[:, 0:128], start=True, stop=True)

            for j in range(NSLOT):
                hT[j] = rpool.tile([128, KHQ, caps[j]], _DT, name=f"hT{j}")
            hsT = rpool.tile([128, KHS, T], _DT)

            # ---- phase B: routed GEMM1 per slot ----
            # Next-phase inputs trigger a few slabs EARLY: the in-order input
            # queue otherwise delivers them just-in-time and the PE stalls.
            w2pre = {}
            for j in range(NSLOT):
                ntc, tsz = dims[j]
                slab = None
                for h in range(KHQ):
                    if (h % 2 == 0 and not (j == 0 and h == 0)) or (j == 0 and h == 1):
                        slab = w1pool.tile([128, KD, 256], _DT)
                        nc.gpsimd.dma_start(slab[:], w1_d[j][h // 2])
                    if h == 5 and j < NSLOT - 1:
                        # next slot's inputs, one slot ahead of use
                        jn = j + 1
                        ntcn, tszn = dims[jn]
                        for i in range(ntcn):
                            xe_sb[jn].append(rpool.tile([128, KD, tszn], _DT, name=f"xe{jn}_{i}"))
                            nc.gpsimd.dma_start(xe_sb[jn][i][:], xe_d[jn][i])
                        b1_sb[jn] = rpool.tile([128, KHQ], mybir.dt.float32, name=f"b1_{jn}")
                        nc.gpsimd.dma_start(b1_sb[jn][:], b1_d[jn][:])
                    if j == NSLOT - 1 and h in (3, 5):
                        # phase-C head: combine weights + first w2 slabs
                        if h == 3:
                            wr_sb[0] = rpool.tile([128, caps[0]], mybir.dt.float32, name="wr0")
                            nc.gpsimd.dma_start(wr_sb[0][:], wr_d[0][:])
                        w2pre[(0, (h - 3) // 2)] = w2pool.tile(
                            [128, KHQ, 256], _DT, name="w2s", tag="w2s")
                        nc.gpsimd.dma_start(w2pre[(0, (h - 3) // 2)][:], w2_d[0][(h - 3) // 2])
                    hh = h % 2
                    for t in range(ntc):
                        ps = pspool.tile([128, 512], mybir.dt.float32)
                        for k in range(KD):
                            nc.tensor.matmul(
                                ps[:, :tsz],
                                w1f[:, k, :] if (j == 0 and h == 0)
                                else slab[:, k, hh * 128:hh * 128 + 128],
                                xe_sb[j][t][:, k, :],
                                start=(k == 0),
                                stop=(k == KD - 1),
                            )
                        nc.scalar.activation(
                            hT[j][:, h, t * tsz:(t + 1) * tsz], ps[:, :tsz], gelu,
                            bias=b1_sb[j][:, h:h + 1],
                        )

            # ---- phase C: routed GEMM2 (tokens moving) + weight scale ----
            ws1_sb = rpool.tile([128, KD, HS], _DT)
            ws2_sb = rpool.tile([128, KHS, D], _DT)
            bs1_sb = rpool.tile([128, KHS], mybir.dt.float32)
            xts_pre = None
            for j in range(NSLOT):
                ntc, tsz = dims[j]
                cap = caps[j]
                for dp in range(4):
                    if (j, dp) in w2pre:
                        w2s = w2pre[(j, dp)]
                    else:
                        w2s = w2pool.tile([128, KHQ, 256], _DT, name="w2s", tag="w2s")
                        nc.gpsimd.dma_start(w2s[:], w2_d[j][dp])
                    # shared-expert loads trickle behind the early slabs
                    if j == 0 and dp == 1:
                        nc.gpsimd.dma_start(ws1_sb[:], ws1_d[:])
                    if j == 0 and dp == 2:
                        nc.gpsimd.dma_start(ws2_sb[:], ws2_d[:])
                    if j == 0 and dp == 3:
                        nc.gpsimd.dma_start(bs1_sb[:], bs1_d[:])
                    if dp == 2 and j < NSLOT - 1:
                        # next slot's phase-C head, two d-slabs ahead
                        jn = j + 1
                        wr_sb[jn] = rpool.tile([128, caps[jn]], mybir.dt.float32, name=f"wr{jn}")
                        nc.gpsimd.dma_start(wr_sb[jn][:], wr_d[jn][:])
                        w2pre[(jn, 0)] = w2pool.tile(
                            [128, KHQ, 256], _DT, name="w2s", tag="w2s")
                        nc.gpsimd.dma_start(w2pre[(jn, 0)][:], w2_d[jn][0])
                    if j == NSLOT - 1 and dp == 0:
                        # phase-D head: first token chunk
                        xts_pre = xtpool.tile([128, KD, 512], _DT, name="xts", tag="xts")
                        nc.gpsimd.dma_start(xts_pre[:], xt_d[0])
                    for dd in range(2):
                        d = 2 * dp + dd
                        eo = opool.tile([128, cap], _DT, name="eo", tag="eo")
                        for t in range(ntc):
                            ps = pspool.tile([128, 512], mybir.dt.float32)
                            for k in range(KHQ):
                                nc.tensor.matmul(
                                    ps[:, :tsz],
                                    w2s[:, k, dd * 128:dd * 128 + 128],
                                    hT[j][:, k, t * tsz:(t + 1) * tsz],
                                    start=(k == 0),
                                    stop=(k == KHQ - 1),
                                )
                            nc.vector.tensor_mul(
                                eo[:, t * tsz:(t + 1) * tsz], ps[:, :tsz],
                                wr_sb[j][:, t * tsz:(t + 1) * tsz],
                            )
                        nc.sync.dma_start(ye_d[j][d * 128:(d + 1) * 128, :], eo[:, :cap])

            # ---- phase D: shared GEMM1 over all T tokens ----
            for tcn in range(4):
                if tcn == 0:
                    xts = xts_pre
                else:
                    xts = xtpool.tile([128, KD, 512], _DT, name="xts", tag="xts")
                    nc.gpsimd.dma_start(xts[:], xt_d[tcn])
                for hs in range(KHS):
                    ps = pspool.tile([128, 512], mybir.dt.float32)
                    for k in range(KD):
                        nc.tensor.matmul(
                            ps[:],
                            ws1_sb[:, k, hs * 128:(hs + 1) * 128],
                            xts[:, k, :],
                            start=(k == 0),
                            stop=(k == KD - 1),
                        )
                    nc.scalar.activation(
                        hsT[:, hs, tcn * 512:(tcn + 1) * 512], ps[:], gelu,
                        bias=bs1_sb[:, hs:hs + 1],
                    )

            # ---- phase E: shared GEMM2 ----
            for t in range(T // 128):
                for dh in range(2):
                    ps = pspool.tile([128, 512], mybir.dt.float32)
                    for k in range(KHS):
                        nc.tensor.matmul(
                            ps[:],
                            hsT[:, k, t * 128:(t + 1) * 128],
                            ws2_sb[:, k, dh * 512:(dh + 1) * 512],
                            start=(k == 0),
                            stop=(k == KHS - 1),
                        )
                    so = opool.tile([128, 512], _DT, tag="so")
                    # the last four tiles alternate scalar/vector casts so the
                    # closing cast+trigger chains run on two queue pairs in
                    # parallel (scalar wakes at t=14, hiding its wake latency)
                    if t >= T // 128 - 2 and dh == 0:
                        nc.scalar.copy(so[:], ps[:])
                        nc.scalar.dma_start(
                            sh_d[t * 128:(t + 1) * 128, dh * 512:(dh + 1) * 512],
                            so[:],
                        )
                    elif t == T // 128 - 1 and dh == 1:
                        # final tile: one cast, then the DMA split across the
                        # Sync and Scalar queues to trigger and drain in parallel
                        nc.vector.tensor_copy(so[:], ps[:])
                        nc.sync.dma_start(
                            sh_d[t * 128:(t + 1) * 128, dh * 512:dh * 512 + 256],
                            so[:, 0:256],
                        )
                        nc.scalar.dma_start(
                            sh_d[t * 128:(t + 1) * 128, dh * 512 + 256:(dh + 1) * 512],
                            so[:, 256:512],
                        )
                    else:
                        nc.vector.tensor_copy(so[:], ps[:])
                        nc.sync.dma_start(
                            sh_d[t * 128:(t + 1) * 128, dh * 512:(dh + 1) * 512],
                            so[:],
                        )

    nc.finalize()
    return nc


def _routing(xf, Wg, bg, bias):
    """Host gate: fp64 for a stable top-2 ranking (matches fp32 reference
    ordering except for ~1e-7-wide ties, which don't occur at these margins)."""
    logits = xf.astype(np.float64) @ Wg.T.astype(np.float64) + bg + bias
    scores = (1.0 / (1.0 + np.exp(-logits))).astype(np.float32)
    # stable sort => ties break toward the lower expert index, like lax.top_k
    top_idx = np.argsort(-scores, axis=1, kind="stable")[:, :TOP_K]
    top_w = np.take_along_axis(scores, top_idx, axis=1)
    return top_idx, top_w


def kernel(x, Wg, bg, bias, W1, b1, W2, b2, Ws1, bs1, Ws2, bs2):
    x = np.asarray(x, F32)
    Wg, bg, bias = np.asarray(Wg, F32), np.asarray(bg, F32), np.asarray(bias, F32)
    W1, b1 = np.asarray(W1, F32), np.asarray(b1, F32)
    W2, b2 = np.asarray(W2, F32), np.asarray(b2, F32)
    Ws1, bs1 = np.asarray(Ws1, F32), np.asarray(bs1, F32)
    Ws2, bs2 = np.asarray(Ws2, F32), np.asarray(bs2, F32)

    xf = x.reshape(-1, D)
    top_idx, top_w = _routing(xf, Wg, bg, bias)

    sels, ws = [], []
    for e in range(E):
        pick = (top_idx == e)
        sel = np.where(pick.any(axis=1))[0]
        w = np.where(pick[sel, 0], top_w[sel, 0], top_w[sel, 1]).astype(F32)
        sels.append(sel)
        ws.append(w)
    counts = np.array([len(s) for s in sels])
    # adjacent pairing of count-sorted experts minimizes the sum of
    # per-slot caps (each slot-class holds two near-equal experts)
    order = np.argsort(-counts, kind="stable")
    pairs = [(int(order[2 * j]), int(order[2 * j + 1])) for j in range(NSLOT)]
    # smallest-cap pair first: slot 0's xe is then a single <=512-token
    # chunk, so the stream start has no second-chunk dependency (which
    # otherwise either delays the start ~2.6us behind a keep-alive or
    # surfaces as a ~1.8us HAM-threatening bubble), and slot 0's slower
    # per-slab consumption (~1.7us/h-block) lets the software-DGE ramp
    # keep the early slab stream continuous
    pairs.sort(key=lambda ab: max(counts[ab[0]], counts[ab[1]]))
    dims = tuple(_slotdims(max(counts[a], counts[b])) for a, b in pairs)

    if dims not in _cache:
        _cache[dims] = _build(dims)
    nc = _cache[dims]

    x_bf = xf.astype(BF16)
    # xt: [4, 128, KD, 512]  (token-chunk major, partition-major inside)
    xt = np.ascontiguousarray(
        x_bf.T.reshape(KD, 128, 4, 512).transpose(2, 1, 0, 3)
    )

    # Quarter-expert weight re-layouts are input-independent; cache across
    # calls (keyed by content hash, so a reused buffer can't serve stale
    # layouts).
    hsh = hashlib.blake2b(digest_size=16)
    for a in (W1, W2, Ws1, Ws2, b1, bs1):
        hsh.update(np.ascontiguousarray(a).data)
    wkey = hsh.hexdigest()
    wmaps = _wcache.get(wkey)
    if wmaps is None:
        wmaps = {"quarter": {}, "core": []}
        for e in range(E):
            for q in range(4):
                r0 = q * HQ
                w1t = (
                    W1[e][r0:r0 + HQ].T.reshape(KD, 128, KHQ // 2, 256)
                    .transpose(2, 1, 0, 3).astype(BF16)
                )
                wmaps["quarter"][(e, q)] = {
                    # W1 quarter rows -> W1qT [D, HQ] -> [4, 128, KD, 256]
                    "w1": np.ascontiguousarray(w1t),
                    "w1f": np.ascontiguousarray(w1t[0][:, :, 0:128]),
                    # W2 quarter cols -> W2qT [HQ, D] -> [4, 128, KHQ, 256]
                    "w2": np.ascontiguousarray(
                        W2[e][:, r0:r0 + HQ].T.reshape(KHQ, 128, 4, 256)
                        .transpose(2, 1, 0, 3).astype(BF16)
                    ),
                    "b1": np.ascontiguousarray(b1[e][r0:r0 + HQ].reshape(KHQ, 128).T),
                }
        for c in range(N_CORES):
            hs0 = c * HS
            wmaps["core"].append({
                "ws1": np.ascontiguousarray(
                    Ws1[hs0:hs0 + HS].T.reshape(KD, 128, HS)
                    .transpose(1, 0, 2).astype(BF16)
                ),
                "ws2": np.ascontiguousarray(
                    Ws2[:, hs0:hs0 + HS].T.reshape(KHS, 128, D)
                    .transpose(1, 0, 2).astype(BF16)
                ),
                "bs1c": np.ascontiguousarray(bs1[hs0:hs0 + HS].reshape(KHS, 128).T),
            })
        _wcache.clear()
        _wcache[wkey] = wmaps

    # per-expert gathered tokens + combine weights at the slot cap
    def gathered(e, ntc, tsz):
        cap = ntc * tsz
        sel, w = sels[e], ws[e]
        xe = np.zeros((cap, D), BF16)
        xe[: len(sel)] = x_bf[sel]
        xe_t = np.ascontiguousarray(
            xe.T.reshape(KD, 128, ntc, tsz).transpose(2, 1, 0, 3)
        )
        wpad = np.zeros(cap, F32)
        wpad[: len(w)] = w
        wr = np.ascontiguousarray(np.broadcast_to(wpad, (128, cap)))
        return xe_t, wr

    gcache = {}
    in_maps = []
    for c in range(N_CORES):
        m = {"xt": xt, **wmaps["core"][c]}
        for j in range(NSLOT):
            e, q = pairs[j][c // 4], c % 4
            if e not in gcache:
                gcache[e] = gathered(e, *dims[j])
            m[f"xe{j}"], m[f"wr{j}"] = gcache[e]
            quart = wmaps["quarter"][(e, q)]
            m[f"w1{j}"], m[f"w2{j}"], m[f"b1{j}"] = quart["w1"], quart["w2"], quart["b1"]
            if j == 0:
                m["w1f"] = quart["w1f"]
        in_maps.append(m)

    res = run_bass_kernel_spmd(nc, in_maps, core_ids=list(range(N_CORES)))

    out = np.zeros((T, D), F32)
    for c in range(N_CORES):
        out += res.results[c]["sh"].astype(F32)
        for j in range(NSLOT):
            e = pairs[j][c // 4]
            sel = sels[e]
            out[sel] += res.results[c][f"ye{j}"][:, : len(sel)].T.astype(F32)
    # biases handled host-side: per-token weighted b2, plus bs2
    wdense = np.zeros((T, E), F32)
    np.put_along_axis(wdense, top_idx, top_w, axis=1)
    out += wdense @ b2
    out += bs2
    return out.reshape(x.shape)


# revision 41
# speedup vs baseline: 1.0194x; 1.0194x over previous
"""DeepSeekMoE kernel for 8 TRN2 NeuronCores.

Sharding: quarter-expert-parallel. Each routed expert's FFN is split in
four along the hidden (H) axis across four cores; experts are paired by
sorted token count (1st with 2nd, 3rd with 4th, ...) so each of the four
SPMD slot-classes carries two near-equal experts and its cap is the pair
max — per-core routed matmul columns drop to 128*sum(slot caps) (~265k),
within ~1% of the theoretical balance floor. Each core also owns a 1/8
H-shard of the shared expert (tensor-parallel).

The tiny gate (sigmoid + top-2 over E=8) runs on host; tokens are gathered
per expert, padded to ntc*tsz equal 16B-aligned chunks (SPMD: one program
for all 8 cores), and shipped pre-transposed so every device-side matmul
contracts over the partition dimension. Each core returns
  ye0..ye3: [D, cap_j]  quarter-expert PARTIAL outputs, combine-weighted
  sh:       [T, D]      shared-expert partial (its H-shard)
Host scatter-adds the 4 quarter partials per expert by token index and
sums the 8 sh partials — the output gather performs the MoE combine; no
on-device collectives needed.

Timing model (measured): the PE streams bf16 matmuls at ~0.42 ns/col when
all INPUT streams ride the software-DGE path (gpsimd dma_start) — the
hardware-DGE input path drags the sustained PE clock from ~2.37GHz down
to ~1.97GHz (P0 power state). Output streams on Sync/Scalar HW-DGE do not
trigger it. LDWEIGHTS is fully hidden. The schedule therefore aims for a
single gapless matmul stream:
  - warmup matmuls start on a gpsimd-memset scratch slice (~7.5us, right
    after the framework preamble) and keep-alive matmuls gated on each
    early transfer hold the HAM clock gate open until real data lands.
  - xe ships token-chunk-major and the first slot's first w1 h-block as
    its own 0.25MB tensor, so the first real matmul needs ~0.78MB landed.
  - every next phase's head inputs (next slot's xe, wr, first w2 slabs,
    first xt chunk) are triggered a few slabs EARLY — the in-order input
    queue otherwise delivers them just-in-time and the PE stalls ~1-2us
    at each transition.
  - the last two tile casts alternate scalar/vector and the final sh tile
    DMA is split across the Sync and Scalar queues.

Compute dtype: bf16 operands, fp32 PSUM accumulation (rel err ~4e-3).
fp8 was evaluated and rejected: e4m3's 3-bit mantissa gives ~5e-2 rel err
on these random-walk GEMM sums (no sqrt(K) averaging), over the 2e-2 gate.
"""

import hashlib
import sys

sys.path.insert(0, "/opt/trn_rl_repo")

import numpy as np
import ml_dtypes

import concourse.bass as bass
import concourse.bacc as bacc
import concourse.mybir as mybir
import concourse.tile as tile
from concourse.bass_utils import run_bass_kernel_spmd

BF16 = ml_dtypes.bfloat16
F32 = np.float32

T, D, E, TOP_K, H = 2048, 1024, 8, 2, 4096
HQ = H // 4          # quarter-expert hidden
HS = H // 8          # shared-expert hidden shard per core
KD = D // 128        # 8  k-chunks over D
KHQ = HQ // 128      # 8  k-chunks over a quarter-expert
KHS = HS // 128      # 4  k-chunks over the shared shard
NSLOT = 4
N_CORES = 8

_DT = mybir.dt.bfloat16
_cache: dict = {}
_wcache: dict = {}


def _slotdims(n):
    """Equal token-chunks: (ntc, tsz) with ntc*tsz >= n, tsz <= 512.
    tsz is a multiple of 8 so bf16 moving-operand rows stay 16B-aligned —
    unaligned rows (e.g. tsz=270) measurably slow the PE stream by ~20%."""
    n = max(64, int(n))
    ntc = -(-n // 512)
    tsz = -(-n // (8 * ntc)) * 8
    return ntc, tsz


def _build(dims):
    """Build + finalize the SPMD device program for a 4-tuple of (ntc,tsz)."""
    nc = bacc.Bacc("TRN2", target_bir_lowering=False, debug=False)

    caps = tuple(ntc * tsz for ntc, tsz in dims)
    xe_d, w1_d, w2_d, b1_d, wr_d, ye_d = {}, {}, {}, {}, {}, {}
    for j, ((ntc, tsz), cap) in enumerate(zip(dims, caps)):
        xe_d[j] = nc.dram_tensor(f"xe{j}", [ntc, 128, KD, tsz], _DT, kind="ExternalInput")
        w1_d[j] = nc.dram_tensor(f"w1{j}", [KHQ // 2, 128, KD, 256], _DT, kind="ExternalInput")
        w2_d[j] = nc.dram_tensor(f"w2{j}", [4, 128, KHQ, 256], _DT, kind="ExternalInput")
        b1_d[j] = nc.dram_tensor(f"b1{j}", [128, KHQ], mybir.dt.float32, kind="ExternalInput")
        wr_d[j] = nc.dram_tensor(f"wr{j}", [128, cap], mybir.dt.float32, kind="ExternalInput")
        ye_d[j] = nc.dram_tensor(f"ye{j}", [D, cap], _DT, kind="ExternalOutput")
    # slot-0 h-block 0 duplicated as its own small tensor: the first real
    # matmul then needs only 0.25MB of weights + one xe chunk
    w1f_d = nc.dram_tensor("w1f", [128, KD, 128], _DT, kind="ExternalInput")
    xt_d = nc.dram_tensor("xt", [4, 128, KD, 512], _DT, kind="ExternalInput")
    ws1_d = nc.dram_tensor("ws1", [128, KD, HS], _DT, kind="ExternalInput")
    ws2_d = nc.dram_tensor("ws2", [128, KHS, D], _DT, kind="ExternalInput")
    bs1_d = nc.dram_tensor("bs1c", [128, KHS], mybir.dt.float32, kind="ExternalInput")
    sh_d = nc.dram_tensor("sh", [T, D], _DT, kind="ExternalOutput")

    gelu = mybir.ActivationFunctionType.Gelu

    with tile.TileContext(nc) as tc:
        with (
            tc.tile_pool(name="resident", bufs=1) as rpool,
            tc.tile_pool(name="w1s", bufs=4) as w1pool,
            tc.tile_pool(name="w2s", bufs=4) as w2pool,
            tc.tile_pool(name="xts", bufs=3) as xtpool,
            tc.tile_pool(name="psum", bufs=7, space="PSUM") as pspool,
            tc.tile_pool(name="wpsum", bufs=1, space="PSUM") as wpspool,
            tc.tile_pool(name="outs", bufs=6) as opool,
        ):
            # ---- PE warmup: dummy matmuls while the first DMAs are in
            # flight, so the HAM clock gate is released for the real work.
            scratch = rpool.tile([128, 512], _DT)
            nc.gpsimd.memset(scratch[:, 0:128], 0.0)
            nc.vector.memset(scratch[:, 128:512], 0.0)
            wps = wpspool.tile([128, 512], mybir.dt.float32)
            for _ in range(2):
                nc.tensor.matmul(wps[:, 0:128], scratch[:, 0:128], scratch[:, 0:128], start=True, stop=True)
            for _ in range(9):
                nc.tensor.matmul(wps[:], scratch[:, 0:128], scratch[:], start=True, stop=True)
            for _ in range(3):
                nc.tensor.matmul(wps[:, 0:128], scratch[:, 0:128], scratch[:, 0:128], start=True, stop=True)

            # ---- slot-0-critical loads, in first-consumption order, all on
            # the gpsimd (software-DGE) queue
            xe_sb = {j: [] for j in range(NSLOT)}
            wr_sb, b1_sb, hT = {}, {}, {}
            (ntc0, tsz0) = dims[0]
            for i in range(ntc0):
                xe_sb[0].append(rpool.tile([128, KD, tsz0], _DT, name=f"xe0_{i}"))
            nc.gpsimd.dma_start(xe_sb[0][0][:], xe_d[0][0])
            w1f = rpool.tile([128, KD, 128], _DT, name="w1f")
            nc.gpsimd.dma_start(w1f[:], w1f_d[:])
            for i in range(1, ntc0):
                nc.gpsimd.dma_start(xe_sb[0][i][:], xe_d[0][i])
            b1_sb[0] = rpool.tile([128, KHQ], mybir.dt.float32, name="b1_0")
            nc.gpsimd.dma_start(b1_sb[0][:], b1_d[0][:])

            # keep-alive touches hold the HAM gate open until data lands
            nc.tensor.matmul(wps[:, 0:128], xe_sb[0][0][:, 0, 0:128],
                             scratch[:, 0:128], start=True, stop=True)
            nc.tensor.matmul(wps[:, 0:128], w1f[:, 0, :],
                             scratch[:, 0:128], start=True, stop=True)
            if ntc0 > 1:
                nc.tensor.matmul(wps[:, 0:128], xe_sb[0][1][:, 0, 0:128],
                                 scratch[:, 0:128], start=True, stop=True)

            for j in range(NSLOT):
                hT[j] = rpool.tile([128, KHQ, caps[j]], _DT, name=f"hT{j}")
            hsT = rpool.tile([128, KHS, T], _DT)

            # ---- phase B: routed GEMM1 per slot ----
            # Next-phase inputs trigger a few slabs EARLY: the in-order input
            # queue otherwise delivers them just-in-time and the PE stalls.
            w2pre = {}
            for j in range(NSLOT):
                ntc, tsz = dims[j]
                slab = None
                for h in range(KHQ):
                    if (h % 2 == 0 and not (j == 0 and h == 0)) or (j == 0 and h == 1):
                        slab = w1pool.tile([128, KD, 256], _DT)
                        nc.gpsimd.dma_start(slab[:], w1_d[j][h // 2])
                    if h == 5 and j < NSLOT - 1:
                        # next slot's inputs, one slot ahead of use
                        jn = j + 1
                        ntcn, tszn = dims[jn]
                        for i in range(ntcn):
                            xe_sb[jn].append(rpool.tile([128, KD, tszn], _DT, name=f"xe{jn}_{i}"))
                            nc.gpsimd.dma_start(xe_sb[jn][i][:], xe_d[jn][i])
                        b1_sb[jn] = rpool.tile([128, KHQ], mybir.dt.float32, name=f"b1_{jn}")
                        nc.gpsimd.dma_start(b1_sb[jn][:], b1_d[jn][:])
                    if j == NSLOT - 1 and h in (3, 5):
                        # phase-C head: combine weights + first w2 slabs
                        if h == 3:
                            wr_sb[0] = rpool.tile([128, caps[0]], mybir.dt.float32, name="wr0")
                            nc.gpsimd.dma_start(wr_sb[0][:], wr_d[0][:])
                        w2pre[(0, (h - 3) // 2)] = w2pool.tile(
                            [128, KHQ, 256], _DT, name="w2s", tag="w2s")
                        nc.gpsimd.dma_start(w2pre[(0, (h - 3) // 2)][:], w2_d[0][(h - 3) // 2])
                    hh = h % 2
                    for t in range(ntc):
                        ps = pspool.tile([128, 512], mybir.dt.float32)
                        for k in range(KD):
                            nc.tensor.matmul(
                                ps[:, :tsz],
                                w1f[:, k, :] if (j == 0 and h == 0)
                                else slab[:, k, hh * 128:hh * 128 + 128],
                                xe_sb[j][t][:, k, :],
                                start=(k == 0),
                                stop=(k == KD - 1),
                            )
                        nc.scalar.activation(
                            hT[j][:, h, t * tsz:(t + 1) * tsz], ps[:, :tsz], gelu,
                            bias=b1_sb[j][:, h:h + 1],
                        )

            # ---- phase C: routed GEMM2 (tokens moving) + weight scale ----
            ws1_sb = rpool.tile([128, KD, HS], _DT)
            ws2_sb = rpool.tile([128, KHS, D], _DT)
            bs1_sb = rpool.tile([128, KHS], mybir.dt.float32)
            xts_pre = None
            for j in range(NSLOT):
                ntc, tsz = dims[j]
                cap = caps[j]
                for dp in range(4):
                    if (j, dp) in w2pre:
                        w2s = w2pre[(j, dp)]
                    else:
                        w2s = w2pool.tile([128, KHQ, 256], _DT, name="w2s", tag="w2s")
                        nc.gpsimd.dma_start(w2s[:], w2_d[j][dp])
                    # shared-expert loads trickle behind the early slabs
                    if j == 0 and dp == 1:
                        nc.gpsimd.dma_start(ws1_sb[:], ws1_d[:])
                    if j == 0 and dp == 2:
                        nc.gpsimd.dma_start(ws2_sb[:], ws2_d[:])
                    if j == 0 and dp == 3:
                        nc.gpsimd.dma_start(bs1_sb[:], bs1_d[:])
                    if dp == 2 and j < NSLOT - 1:
                        # next slot's phase-C head, two d-slabs ahead
                        jn = j + 1
                        wr_sb[jn] = rpool.tile([128, caps[jn]], mybir.dt.float32, name=f"wr{jn}")
                        nc.gpsimd.dma_start(wr_sb[jn][:], wr_d[jn][:])
                        w2pre[(jn, 0)] = w2pool.tile(
                            [128, KHQ, 256], _DT, name="w2s", tag="w2s")
                        nc.gpsimd.dma_start(w2pre[(jn, 0)][:], w2_d[jn][0])
                    if j == NSLOT - 1 and dp == 0:
                        # phase-D head: first token chunk
                        xts_pre = xtpool.tile([128, KD, 512], _DT, name="xts", tag="xts")
                        nc.gpsimd.dma_start(xts_pre[:], xt_d[0])
                    for dd in range(2):
                        d = 2 * dp + dd
                        eo = opool.tile([128, cap], _DT, name="eo", tag="eo")
                        for t in range(ntc):
                            ps = pspool.tile([128, 512], mybir.dt.float32)
                            for k in range(KHQ):
                                nc.tensor.matmul(
                                    ps[:, :tsz],
                                    w2s[:, k, dd * 128:dd * 128 + 128],
                                    hT[j][:, k, t * tsz:(t + 1) * tsz],
                                    start=(k == 0),
                                    stop=(k == KHQ - 1),
                                )
                            nc.vector.tensor_mul(
                                eo[:, t * tsz:(t + 1) * tsz], ps[:, :tsz],
                                wr_sb[j][:, t * tsz:(t + 1) * tsz],
                            )
                        nc.sync.dma_start(ye_d[j][d * 128:(d + 1) * 128, :], eo[:, :cap])

            # ---- phase D: shared GEMM1 over all T tokens ----
            for tcn in range(4):
                if tcn == 0:
                    xts = xts_pre
                else:
                    xts = xtpool.tile([128, KD, 512], _DT, name="xts", tag="xts")
                    nc.gpsimd.dma_start(xts[:], xt_d[tcn])
                for hs in range(KHS):
                    ps = pspool.tile([128, 512], mybir.dt.float32)
                    for k in range(KD):
                        nc.tensor.matmul(
                            ps[:],
                            ws1_sb[:, k, hs * 128:(hs + 1) * 128],
                            xts[:, k, :],
                            start=(k == 0),
                            stop=(k == KD - 1),
                        )
                    nc.scalar.activation(
                        hsT[:, hs, tcn * 512:(tcn + 1) * 512], ps[:], gelu,
                        bias=bs1_sb[:, hs:hs + 1],
                    )

            # ---- phase E: shared GEMM2 ----
            for t in range(T // 128):
                for dh in range(2):
                    if t == T // 128 - 1 and dh == 1:
                        # final tile as two independent 256-col pieces, piece0
                        # on the scalar queue pair and piece1 on vector+Sync:
                        # the closing drain is a 256-col cast + one trigger +
                        # half the packets, with piece0's chain fully parallel
                        for c in range(2):
                            ps = pspool.tile([128, 512], mybir.dt.float32)
                            c0 = dh * 512 + c * 256
                            for k in range(KHS):
                                nc.tensor.matmul(
                                    ps[:, 0:256],
                                    hsT[:, k, t * 128:(t + 1) * 128],
                                    ws2_sb[:, k, c0:c0 + 256],
                                    start=(k == 0),
                                    stop=(k == KHS - 1),
                                )
                            so = opool.tile([128, 512], _DT, tag="so")
                            if c == 0:
                                nc.scalar.copy(so[:, 0:256], ps[:, 0:256])
                                nc.scalar.dma_start(
                                    sh_d[t * 128:(t + 1) * 128, c0:c0 + 256],
                                    so[:, 0:256],
                                )
                            else:
                                nc.vector.tensor_copy(so[:, 0:256], ps[:, 0:256])
                                nc.sync.dma_start(
                                    sh_d[t * 128:(t + 1) * 128, c0:c0 + 256],
                                    so[:, 0:256],
                                )
                        continue
                    ps = pspool.tile([128, 512], mybir.dt.float32)
                    for k in range(KHS):
                        nc.tensor.matmul(
                            ps[:],
                            hsT[:, k, t * 128:(t + 1) * 128],
                            ws2_sb[:, k, dh * 512:(dh + 1) * 512],
                            start=(k == 0),
                            stop=(k == KHS - 1),
                        )
                    so = opool.tile([128, 512], _DT, tag="so")
                    # the last four tiles alternate scalar/vector casts so the
                    # closing cast+trigger chains run on two queue pairs in
                    # parallel (scalar wakes at t=14, hiding its wake latency)
                    if t >= T // 128 - 2 and dh == 0:
                        nc.scalar.copy(so[:], ps[:])
                        nc.scalar.dma_start(
                            sh_d[t * 128:(t + 1) * 128, dh * 512:(dh + 1) * 512],
                            so[:],
                        )
                    else:
                        nc.vector.tensor_copy(so[:], ps[:])
                        nc.sync.dma_start(
                            sh_d[t * 128:(t + 1) * 128, dh * 512:(dh + 1) * 512],
                            so[:],
                        )

    nc.finalize()
    return nc


def _routing(xf, Wg, bg, bias):
    """Host gate: fp64 for a stable top-2 ranking (matches fp32 reference
    ordering except for ~1e-7-wide ties, which don't occur at these margins)."""
    logits = xf.astype(np.float64) @ Wg.T.astype(np.float64) + bg + bias
    scores = (1.0 / (1.0 + np.exp(-logits))).astype(np.float32)
    # stable sort => ties break toward the lower expert index, like lax.top_k
    top_idx = np.argsort(-scores, axis=1, kind="stable")[:, :TOP_K]
    top_w = np.take_along_axis(scores, top_idx, axis=1)
    return top_idx, top_w


def kernel(x, Wg, bg, bias, W1, b1, W2, b2, Ws1, bs1, Ws2, bs2):
    x = np.asarray(x, F32)
    Wg, bg, bias = np.asarray(Wg, F32), np.asarray(bg, F32), np.asarray(bias, F32)
    W1, b1 = np.asarray(W1, F32), np.asarray(b1, F32)
    W2, b2 = np.asarray(W2, F32), np.asarray(b2, F32)
    Ws1, bs1 = np.asarray(Ws1, F32), np.asarray(bs1, F32)
    Ws2, bs2 = np.asarray(Ws2, F32), np.asarray(bs2, F32)

    xf = x.reshape(-1, D)
    top_idx, top_w = _routing(xf, Wg, bg, bias)

    sels, ws = [], []
    for e in range(E):
        pick = (top_idx == e)
        sel = np.where(pick.any(axis=1))[0]
        w = np.where(pick[sel, 0], top_w[sel, 0], top_w[sel, 1]).astype(F32)
        sels.append(sel)
        ws.append(w)
    counts = np.array([len(s) for s in sels])
    # adjacent pairing of count-sorted experts minimizes the sum of
    # per-slot caps (each slot-class holds two near-equal experts)
    order = np.argsort(-counts, kind="stable")
    pairs = [(int(order[2 * j]), int(order[2 * j + 1])) for j in range(NSLOT)]
    dims = tuple(_slotdims(max(counts[a], counts[b])) for a, b in pairs)

    if dims not in _cache:
        _cache[dims] = _build(dims)
    nc = _cache[dims]

    x_bf = xf.astype(BF16)
    # xt: [4, 128, KD, 512]  (token-chunk major, partition-major inside)
    xt = np.ascontiguousarray(
        x_bf.T.reshape(KD, 128, 4, 512).transpose(2, 1, 0, 3)
    )

    # Quarter-expert weight re-layouts are input-independent; cache across
    # calls (keyed by content hash, so a reused buffer can't serve stale
    # layouts).
    hsh = hashlib.blake2b(digest_size=16)
    for a in (W1, W2, Ws1, Ws2, b1, bs1):
        hsh.update(np.ascontiguousarray(a).data)
    wkey = hsh.hexdigest()
    wmaps = _wcache.get(wkey)
    if wmaps is None:
        wmaps = {"quarter": {}, "core": []}
        for e in range(E):
            for q in range(4):
                r0 = q * HQ
                w1t = (
                    W1[e][r0:r0 + HQ].T.reshape(KD, 128, KHQ // 2, 256)
                    .transpose(2, 1, 0, 3).astype(BF16)
                )
                wmaps["quarter"][(e, q)] = {
                    # W1 quarter rows -> W1qT [D, HQ] -> [4, 128, KD, 256]
                    "w1": np.ascontiguousarray(w1t),
                    "w1f": np.ascontiguousarray(w1t[0][:, :, 0:128]),
                    # W2 quarter cols -> W2qT [HQ, D] -> [4, 128, KHQ, 256]
                    "w2": np.ascontiguousarray(
                        W2[e][:, r0:r0 + HQ].T.reshape(KHQ, 128, 4, 256)
                        .transpose(2, 1, 0, 3).astype(BF16)
                    ),
                    "b1": np.ascontiguousarray(b1[e][r0:r0 + HQ].reshape(KHQ, 128).T),
                }
        for c in range(N_CORES):
            hs0 = c * HS
            wmaps["core"].append({
                "ws1": np.ascontiguousarray(
                    Ws1[hs0:hs0 + HS].T.reshape(KD, 128, HS)
                    .transpose(1, 0, 2).astype(BF16)
                ),
                "ws2": np.ascontiguousarray(
                    Ws2[:, hs0:hs0 + HS].T.reshape(KHS, 128, D)
                    .transpose(1, 0, 2).astype(BF16)
                ),
                "bs1c": np.ascontiguousarray(bs1[hs0:hs0 + HS].reshape(KHS, 128).T),
            })
        _wcache.clear()
        _wcache[wkey] = wmaps

    # per-expert gathered tokens + combine weights at the slot cap
    def gathered(e, ntc, tsz):
        cap = ntc * tsz
        sel, w = sels[e], ws[e]
        xe = np.zeros((cap, D), BF16)
        xe[: len(sel)] = x_bf[sel]
        xe_t = np.ascontiguousarray(
            xe.T.reshape(KD, 128, ntc, tsz).transpose(2, 1, 0, 3)
        )
        wpad = np.zeros(cap, F32)
        wpad[: len(w)] = w
        wr = np.ascontiguousarray(np.broadcast_to(wpad, (128, cap)))
        return xe_t, wr

    gcache = {}
    in_maps = []
    for c in range(N_CORES):
        m = {"xt": xt, **wmaps["core"][c]}
        for j in range(NSLOT):
            e, q = pairs[j][c // 4], c % 4
            if e not in gcache:
                gcache[e] = gathered(e, *dims[j])
            m[f"xe{j}"], m[f"wr{j}"] = gcache[e]
            quart = wmaps["quarter"][(e, q)]
            m[f"w1{j}"], m[f"w2{j}"], m[f"b1{j}"] = quart["w1"], quart["w2"], quart["b1"]
            if j == 0:
                m["w1f"] = quart["w1f"]
        in_maps.append(m)

    res = run_bass_kernel_spmd(nc, in_maps, core_ids=list(range(N_CORES)))

    out = np.zeros((T, D), F32)
    for c in range(N_CORES):
        out += res.results[c]["sh"].astype(F32)
        for j in range(NSLOT):
            e = pairs[j][c // 4]
            sel = sels[e]
            out[sel] += res.results[c][f"ye{j}"][:, : len(sel)].T.astype(F32)
    # biases handled host-side: per-token weighted b2, plus bs2
    wdense = np.zeros((T, E), F32)
    np.put_along_axis(wdense, top_idx, top_w, axis=1)
    out += wdense @ b2
    out += bs2
    return out.reshape(x.shape)
